# revision 2
# baseline (speedup 1.0000x reference)
"""DepthCueExtractor TRN2 kernel.

out[b,u,y,x,n] = mean_v(lfi[b,u,y,x,v]) * s_mask[b,n] * h_mask[b,n,y]
  s_mask[b,n]   = sum_{h,w} f_maps[b,h,w,n]
  h_mask[b,n,y] = colsum[b,y,n] / max_w colsum[b,w,n]
  colsum[b,w,n] = sum_h f_maps[b,h,w,n]

Sharding: 8 cores = (batch b in 0..3) x (H-half in 0..1), data-parallel on the
output; cores are fully independent (no collectives -- the 15us constant
overhead of a collective can no longer hide under the shortened DMA stream).
Each core reads its lfi slice (cast f32->bf16 in the DMA) plus the full
f_maps[b] (cast f32->fp8e4m3 in the DMA; the host rotates the W axis per core
so "my" 128 colsum rows are always the first block, keeping the SPMD program
identical across cores). colsum is reduced over the H partition dim with fp8
PE ones-matmuls; the per-sample stats (sum / max over w) use gpsimd partition
reduces. The output phase writes bf16 (rel-err ~2^-9, far inside the 2e-2
gate) and the broadcast multiply is spread across three engines so it hides
under the DMA stream: per u-group of four 1MB tiles, one direct DVE
tensor_mul, two Act-assisted tiles (scalar engine materializes the mlf
broadcast as packed bf16, DVE then runs the mul in 2x_1p mode), and one on
gpsimd. ~47.3MB of DMA traffic per core (4.2 fm + 5.3 lfi + 37.7 store),
~131us of serialized DMA-engine time."""

import numpy as np

import concourse.bass as bass
import concourse.bacc as bacc
import concourse.bass_isa as bass_isa
import concourse.mybir as mybir
import concourse.tile as tile
from concourse.bass_utils import run_bass_kernel_spmd

F32 = mybir.dt.float32
BF16 = mybir.dt.bfloat16
FP8 = mybir.dt.float8e4

B, U, H, W, V, N = 4, 9, 256, 256, 9, 64
HY = H // 2


def build_kernel_body(nc, tc, lfi_s, fm, out_s):
    with (
        tc.tile_pool(name="const", bufs=1) as const_pool,
        tc.tile_pool(name="fmp", bufs=1) as fm_pool,
        tc.tile_pool(name="psum", bufs=1, space="PSUM") as psum_pool,
        tc.tile_pool(name="stats", bufs=1) as stats_pool,
        tc.tile_pool(name="lfip", bufs=6) as lfi_pool,
        tc.tile_pool(name="mlfp", bufs=1) as mlf_pool,
        tc.tile_pool(name="outp", bufs=2) as out_pool,
    ):
        ones = const_pool.tile([128, 1], FP8)
        nc.vector.memset(ones[:], 1.0)

        # ---- loads (all on the gpsimd SWDGE queue, which can cast) --------
        # fm h-halves, f32 -> fp8
        fmh = []
        for h in range(2):
            t = fm_pool.tile([128, W, N], FP8, name=f"fmh{h}")
            nc.gpsimd.dma_start(out=t[:], in_=fm[h * HY : (h + 1) * HY, :, :])
            fmh.append(t)
        # first four lfi u-slices, f32 -> bf16
        lts = []
        for u in range(U):
            lt = lfi_pool.tile([128, W, V], BF16, name=f"lt{u}", tag="lt", bufs=6)
            lts.append(lt)
        for u in range(4):
            nc.gpsimd.dma_start(out=lts[u][:], in_=lfi_s[u])

        # ---- colsum[w, n] = sum_h fm[h, w, n] via fp8 PE ones-matmuls -----
        cs_ps = []
        for wc in range(2):
            ps = psum_pool.tile([128, N], F32, name=f"cs_ps{wc}")
            for n in range(N):
                for h in range(2):
                    nc.tensor.matmul(
                        out=ps[:, n : n + 1],
                        lhsT=fmh[h][:, wc * 128 : (wc + 1) * 128, n],
                        rhs=ones[:, 0:1],
                        start=(h == 0),
                        stop=(h == 1),
                    )
            cs_ps.append(ps)

        csA = stats_pool.tile([128, N], F32)
        nc.vector.tensor_copy(out=csA[:], in_=cs_ps[0][:])
        csB = stats_pool.tile([128, N], F32)
        nc.vector.tensor_copy(out=csB[:], in_=cs_ps[1][:])

        # ---- stats: s = sum_w colsum, m = max_w colsum (gpsimd) -----------
        raA = stats_pool.tile([128, N], F32)
        nc.gpsimd.partition_all_reduce(raA[:], csA[:], 128, bass_isa.ReduceOp.add)
        raB = stats_pool.tile([128, N], F32)
        nc.gpsimd.partition_all_reduce(raB[:], csB[:], 128, bass_isa.ReduceOp.add)
        rmA = stats_pool.tile([128, N], F32)
        nc.gpsimd.partition_all_reduce(rmA[:], csA[:], 128, bass_isa.ReduceOp.max)
        rmB = stats_pool.tile([128, N], F32)
        nc.gpsimd.partition_all_reduce(rmB[:], csB[:], 128, bass_isa.ReduceOp.max)

        # remaining lfi loads (after the stat reduces in the Pool SEQ stream
        # so their descriptor-gen does not delay the wf critical path)
        for u in range(4, U):
            nc.gpsimd.dma_start(out=lts[u][:], in_=lfi_s[u])

        # ---- finalize wf[y, n] = colsum[y,n] * s[n] / (max[n] * V) --------
        s_all = stats_pool.tile([128, N], F32)
        nc.vector.tensor_add(out=s_all[:], in0=raA[:], in1=raB[:])
        m_all = stats_pool.tile([128, N], F32)
        nc.vector.tensor_max(out=m_all[:], in0=rmA[:], in1=rmB[:])
        m9 = stats_pool.tile([128, N], F32)
        nc.vector.tensor_scalar_mul(m9[:], m_all[:], float(V))
        rec = stats_pool.tile([128, N], F32)
        nc.vector.reciprocal(out=rec[:], in_=m9[:])
        sn = stats_pool.tile([128, N], F32)
        nc.vector.tensor_mul(out=sn[:], in0=s_all[:], in1=rec[:])
        wf = stats_pool.tile([128, N], F32)
        nc.vector.tensor_mul(out=wf[:], in0=csA[:], in1=sn[:])
        wfb = stats_pool.tile([128, N], BF16)
        nc.vector.tensor_copy(out=wfb[:], in_=wf[:])

        # ---- output phase -------------------------------------------------
        mlf = [
            mlf_pool.tile([128, W], F32, name=f"mlf{u}", tag=f"mlf{u}")
            for u in range(U)
        ]

        def reduce_u(u):
            nc.vector.reduce_sum(
                out=mlf[u][:], in_=lts[u][:], axis=mybir.AxisListType.X
            )

        xw = 64

        def mlf_bcast(u, x0):
            msl = mlf[u][:, x0 : x0 + xw]
            return bass.AP(
                tensor=msl.tensor, offset=msl.offset, ap=list(msl.ap) + [[0, N]]
            )

        def wf_bcast(t):
            return bass.AP(
                tensor=t.tensor, offset=t.offset, ap=[t.ap[0], [0, xw], t.ap[1]]
            )

        def store(u, x0, ot):
            nc.sync.dma_start(out=out_s[u, :, x0 : x0 + xw, :], in_=ot[:])

        def emit_direct(u, x0):
            ot = out_pool.tile([128, xw, N], BF16, name=f"otd{u}_{x0}", tag="otd", bufs=3)
            nc.vector.tensor_mul(out=ot[:], in0=mlf_bcast(u, x0), in1=wf_bcast(wf))
            store(u, x0, ot)

        def emit_act_copy(u, x0):
            mr = out_pool.tile([128, xw, N], BF16, name=f"mr{u}_{x0}", tag="mr", bufs=3)
            nc.scalar.copy(out=mr[:], in_=mlf_bcast(u, x0))
            return mr

        def emit_act_mul(u, x0, mr):
            ot = out_pool.tile([128, xw, N], BF16, name=f"ota{u}_{x0}", tag="ota", bufs=3)
            nc.vector.tensor_mul(out=ot[:], in0=mr[:], in1=wf_bcast(wfb))
            store(u, x0, ot)

        def emit_pool(u, x0):
            ot = out_pool.tile([128, xw, N], BF16, name=f"otp{u}_{x0}", tag="otp", bufs=3)
            nc.gpsimd.tensor_mul(out=ot[:], in0=mlf_bcast(u, x0), in1=wf_bcast(wf))
            store(u, x0, ot)

        reduce_u(0)
        for u in range(U):
            mr1 = emit_act_copy(u, 1 * xw)
            mr2 = emit_act_copy(u, 2 * xw)
            emit_pool(u, 3 * xw)
            if u + 1 < U:
                reduce_u(u + 1)
            emit_direct(u, 0)
            emit_act_mul(u, 1 * xw, mr1)
            emit_act_mul(u, 2 * xw, mr2)


def build_nc():
    nc = bacc.Bacc("TRN2", target_bir_lowering=False, debug=True)
    lfi_s = nc.dram_tensor("lfi_s", [U, HY, W, V], F32, kind="ExternalInput")
    fm = nc.dram_tensor("fm", [H, W, N], F32, kind="ExternalInput")
    out_s = nc.dram_tensor("out_s", [U, HY, W, N], BF16, kind="ExternalOutput")
    with tile.TileContext(nc) as tc:
        build_kernel_body(nc, tc, lfi_s, fm, out_s)
    nc.compile()
    return nc


_CACHE = {}


def make_in_maps(lfi, f_maps):
    in_maps = []
    for c in range(8):
        b, half = divmod(c, 2)
        lf = np.ascontiguousarray(lfi[b, :, half * HY : (half + 1) * HY])
        fb = f_maps[b]
        if half == 0:
            fmc = np.ascontiguousarray(fb)
        else:
            # rotate W so this core's own colsum rows are the first 128
            fmc = np.ascontiguousarray(
                np.concatenate([fb[:, HY:, :], fb[:, :HY, :]], axis=1)
            )
        in_maps.append({"lfi_s": lf, "fm": fmc})
    return in_maps


def kernel(lfi, f_maps):
    lfi = np.asarray(lfi, dtype=np.float32)
    f_maps = np.asarray(f_maps, dtype=np.float32)
    if "nc" not in _CACHE:
        _CACHE["nc"] = build_nc()
    nc = _CACHE["nc"]
    res = run_bass_kernel_spmd(nc, make_in_maps(lfi, f_maps), list(range(8)))
    out = np.empty((B, U, H, W, N), np.float32)
    for c in range(8):
        b, half = divmod(c, 2)
        out[b, :, half * HY : (half + 1) * HY] = np.asarray(
            res.results[c]["out_s"]
        ).astype(np.float32)
    return out
